# revision 17
# baseline (speedup 1.0000x reference)
"""MoE group-limited routing gate (DeepSeek-style) on 8 Trainium2 NeuronCores.

Computation (per token t over E=256 experts, D=7168 features):
    logits = x @ weight.T                      [T, E]
    group-limited top-k: 8 groups of 32 experts, keep top-4 groups by
    group-max, then top-8 experts among kept groups.
    weights = sigmoid(logits[sel]) normalized to sum 1, * 2.5
Returns (weights [T,8] f32, indices [T,8] int32) like the reference.

Strategy: data-parallel over tokens, 2048 tokens/core, gate weight
replicated.  Matmul runs as one fp16 pass plus two fp8-e4m3 passes in
DoubleRow mode (2 k-chunks per instruction at 0.5 cycles/row), all
accumulating into a single PSUM tile via power-of-2 scale alignment:

    psum = xh @ (w<<16)_fp16  +  (xl<<11)_fp8 @ (w<<5)_fp8
                              +  (xh)_fp8 @ ((w - wh)<<16)_fp8
    logits = psum * 2^-16

fp16/fp8 products are exact in the f32 PSUM accumulator; the remaining
error is the fp8 rounding of the correction terms (~2e-5 absolute on
logits whose std is ~2.0), flipping ~7 of 131072 top-k slots.

Host pre-transposes into [128, block, chunk, token] layout so every DMA
lands 3.5-7KB contiguous runs per partition (vs 256-512B for a naive
[D, T] layout).  Top-k runs directly on the scaled PSUM (selection is
scale-invariant); the 2^-16 unscale rides the sigmoid activation's
scale input for free.
"""

import os
import numpy as np
from contextlib import ExitStack

import ml_dtypes

import concourse.bacc as bacc
import concourse.tile as tile
from concourse import mybir
from concourse.bass_utils import run_bass_kernel_spmd

N_CORES = 8
T_FULL = 16384
D = 7168
E = 256
G = 8            # expert groups
EPG = E // G     # experts per group = 32
TOPK = 8
TOPK_GROUPS = 4
ROUTE_SCALE = 2.5

P = 128
T = T_FULL // N_CORES       # 2048 tokens per core
KC = D // P                 # 56 contraction chunks
TB = 256                    # tokens per block
NB = T // TB                # 8 blocks per core
NBF = T_FULL // TB          # 64 blocks total
TPB = TB // P               # 2 token-tiles per block
# DMA split sizes (in k-chunks) per block / weight tensor.  All even so
# DoubleRow chunk pairs never straddle a split.  The first splits are tiny
# so the PE's first matmul can start ~6us into the kernel instead of
# waiting for a full 1MB split.
SPLITS = [2, 6, 8, 8, 8, 8, 8, 8]
SOFF = [sum(SPLITS[:i]) for i in range(len(SPLITS))]
KQ = len(SPLITS)
CH2SPLIT = [(q, o) for q, sz in enumerate(SPLITS) for o in range(sz)]
NEG = -1.0e30
S16 = 2.0 ** 16             # scale of the PSUM accumulator
XL_SCALE = 2.0 ** 11        # xl fp8 pre-scale
W8_SCALE = 2.0 ** 5         # w8 fp8 pre-scale  (XL_SCALE*W8_SCALE == S16)
F8NP = ml_dtypes.float8_e4m3

PRECISION = os.environ.get("KPREC", "hybrid8")

_CACHE = {}


def _emit_topk(nc, sc_pool, out_pool, scores, wout, iout, t0, scale=1.0,
               out_eng=None):
    """Group-limited top-k + normalize on a [128, 256] f32 logits tile.

    scores may hold logits * (1/scale); selection is scale-invariant and
    the unscale is folded into the sigmoid activation."""
    f32 = mybir.dt.float32
    scores_g = scores.rearrange("p (g e) -> p g e", g=G)
    glog = sc_pool.tile([P, G], f32)
    nc.vector.reduce_max(out=glog, in_=scores_g, axis=mybir.AxisListType.X)
    gsort = sc_pool.tile([P, G], f32)
    nc.vector.max(out=gsort, in_=glog)
    # additive mask: 0 for kept groups (>= 4th-largest), -1e30 otherwise
    maskadd = sc_pool.tile([P, G], f32)
    nc.vector.tensor_scalar(
        out=maskadd,
        in0=glog,
        scalar1=gsort[:, TOPK_GROUPS - 1:TOPK_GROUPS],
        scalar2=NEG,
        op0=mybir.AluOpType.is_lt,
        op1=mybir.AluOpType.mult,
    )
    masked = sc_pool.tile([P, E], f32)
    nc.vector.tensor_add(
        masked.rearrange("p (g e) -> p g e", g=G),
        scores_g,
        maskadd.to_broadcast([P, G, EPG]),
    )
    top8 = sc_pool.tile([P, TOPK], f32)
    nc.vector.max(out=top8, in_=masked)
    idx = out_pool.tile([P, TOPK], mybir.dt.uint32)
    nc.vector.max_index(out=idx, in_max=top8, in_values=masked)
    sig = sc_pool.tile([P, TOPK], f32)
    nc.scalar.activation(
        out=sig, in_=top8, func=mybir.ActivationFunctionType.Sigmoid, scale=scale
    )
    ssum = sc_pool.tile([P, 1], f32)
    nc.vector.reduce_sum(out=ssum, in_=sig, axis=mybir.AxisListType.X)
    rec = sc_pool.tile([P, 1], f32)
    nc.vector.reciprocal(out=rec, in_=ssum)
    wres = out_pool.tile([P, TOPK], f32)
    nc.vector.tensor_scalar(
        out=wres,
        in0=sig,
        scalar1=rec[:, 0:1],
        scalar2=ROUTE_SCALE,
        op0=mybir.AluOpType.mult,
        op1=mybir.AluOpType.mult,
    )
    # outputs ride the SWDGE ring so the tiny writes never stall the
    # HWDGE ring that streams x; the last block uses the (by then idle)
    # sync HWDGE ring instead so the SWDGE drain overlaps compute.
    eng = out_eng if out_eng is not None else nc.gpsimd
    eng.dma_start(out=wout[t0:t0 + P, :], in_=wres)
    eng.dma_start(out=iout[t0:t0 + P, :], in_=idx)


def _build_hybrid8():
    nc = bacc.Bacc("TRN2", target_bir_lowering=False, debug=False, num_devices=N_CORES)
    f32 = mybir.dt.float32
    f16 = mybir.dt.float16
    f8 = mybir.dt.float8e4
    DR = mybir.MatmulPerfMode.DoubleRow

    xh = nc.dram_tensor("xh", [P, NB, KC, TB], f16, kind="ExternalInput").ap()
    xl = nc.dram_tensor("xl", [P, NB, KC, TB], f8, kind="ExternalInput").ap()
    x8 = nc.dram_tensor("x8", [P, NB, KC, TB], f8, kind="ExternalInput").ap()
    wh = nc.dram_tensor("wh", [P, KC, E], f16, kind="ExternalInput").ap()
    w8 = nc.dram_tensor("w8", [P, KC, E], f8, kind="ExternalInput").ap()
    wl = nc.dram_tensor("wl", [P, KC, E], f8, kind="ExternalInput").ap()
    wout = nc.dram_tensor("w_out", [T, TOPK], f32, kind="ExternalOutput").ap()
    iout = nc.dram_tensor("i_out", [T, TOPK], mybir.dt.uint32, kind="ExternalOutput").ap()

    with tile.TileContext(nc) as tc, ExitStack() as ctx:
        wt_pool = ctx.enter_context(tc.tile_pool(name="wt", bufs=1))
        xh_pool = ctx.enter_context(tc.tile_pool(name="xh", bufs=2))
        xc_pool = ctx.enter_context(tc.tile_pool(name="xc", bufs=2))
        psum_pool = ctx.enter_context(tc.tile_pool(name="psum", bufs=8, space="PSUM"))
        sc_pool = ctx.enter_context(tc.tile_pool(name="scratch", bufs=3))
        out_pool = ctx.enter_context(tc.tile_pool(name="outs", bufs=4))

        wh_sb, w8_sb, wl_sb = [], [], []

        # weights ride the Activation-engine HWDGE queue, in parallel with
        # the x streams on the sync queue: the prologue (weights + first two
        # x blocks) is DMA-bandwidth-bound, so two rings halve the ramp.
        def load_w(lst, src, dt_, name, q):
            t = wt_pool.tile([P, SPLITS[q], E], dt_, tag=f"{name}{q}")
            nc.scalar.dma_start(out=t, in_=src[:, SOFF[q]:SOFF[q] + SPLITS[q], :])
            lst.append(t)

        xh_blk, xl_blk, x8_blk = {}, {}, {}

        def load_x_split(dst, pool, src, dt_, name, b, q):
            t = pool.tile([P, SPLITS[q], TB], dt_, tag=f"{name}{q}")
            nc.sync.dma_start(out=t, in_=src[:, b, SOFF[q]:SOFF[q] + SPLITS[q], :])
            dst.setdefault(b, []).append(t)

        def load_x(dst, pool, src, dt_, name, b):
            for q in range(KQ):
                load_x_split(dst, pool, src, dt_, name, b, q)

        # DMA emission order == per-queue HWDGE consumption order.
        # Weight queue: wh, w8, wl.  x queue: xh(0), xh(1), xl(0), x8(0),
        # then steady-state per-block inside the loop.
        for q in range(KQ):
            load_w(wh_sb, wh, f16, "wh", q)
        for q in range(KQ):
            load_w(w8_sb, w8, f8, "w8", q)
        for q in range(KQ):
            load_w(wl_sb, wl, f8, "wl", q)
        load_x(xh_blk, xh_pool, xh, f16, "xh", 0)
        load_x(xh_blk, xh_pool, xh, f16, "xh", 1)
        load_x(xl_blk, xc_pool, xl, f8, "xl", 0)
        load_x(x8_blk, xc_pool, x8, f8, "x8", 0)

        psums = {}

        def p1(b):
            ps_list = []
            for j in range(TPB):
                js = slice(j * P, (j + 1) * P)
                ps = psum_pool.tile([P, E], f32)
                for k in range(KC):
                    sp, so = CH2SPLIT[k]
                    nc.tensor.matmul(
                        ps,
                        xh_blk[b][sp][:, so, js],
                        wh_sb[sp][:, so, :],
                        start=(k == 0),
                        stop=False,
                    )
                ps_list.append(ps)
            psums[b] = ps_list

        def corr(b):
            for j in range(TPB):
                js = slice(j * P, (j + 1) * P)
                ps = psums[b][j]
                for q in range(KC // 2):
                    sp, so = CH2SPLIT[2 * q]
                    nc.tensor.matmul(
                        ps,
                        xl_blk[b][sp][:, so:so + 2, js],
                        w8_sb[sp][:, so:so + 2, :],
                        start=False,
                        stop=False,
                        perf_mode=DR,
                    )
                for q in range(KC // 2):
                    sp, so = CH2SPLIT[2 * q]
                    nc.tensor.matmul(
                        ps,
                        x8_blk[b][sp][:, so:so + 2, js],
                        wl_sb[sp][:, so:so + 2, :],
                        start=False,
                        stop=(q == KC // 2 - 1),
                        perf_mode=DR,
                    )
                _emit_topk(
                    nc, sc_pool, out_pool, ps, wout, iout, b * TB + j * P,
                    scale=1.0 / S16,
                    out_eng=nc.sync if b == NB - 1 else None,
                )

        # software pipeline: P1(b+1) runs while block b's correction
        # streams land; corrections of b run while xh(b+2) lands.
        p1(0)
        for b in range(NB):
            if b + 1 < NB:
                if b + 2 < NB:
                    load_x(xh_blk, xh_pool, xh, f16, "xh", b + 2)
                load_x(xl_blk, xc_pool, xl, f8, "xl", b + 1)
                load_x(x8_blk, xc_pool, x8, f8, "x8", b + 1)
                p1(b + 1)
            corr(b)
    nc.compile()
    return nc


def _get_program(precision):
    key = f"nc_{precision}"
    if key not in _CACHE:
        _CACHE[key] = _build_hybrid8()
    return _CACHE[key]


def _xlayout(a, c):
    """[D, T_FULL] -> per-core [P, NB, KC, TB] (d = k*P + p, t = b*TB + tt)."""
    return np.ascontiguousarray(
        a.reshape(KC, P, NBF, TB)[:, :, c * NB:(c + 1) * NB, :].transpose(1, 2, 0, 3)
    )


def _wlayout(a):
    """[D, E] -> [P, KC, E]."""
    return np.ascontiguousarray(a.reshape(KC, P, E).transpose(1, 0, 2))


def kernel(x: np.ndarray, weight: np.ndarray, _trace: bool = False, **_kw):
    x = np.asarray(x, dtype=np.float32)
    weight = np.asarray(weight, dtype=np.float32)
    assert x.shape == (T_FULL, D) and weight.shape == (E, D)

    nc = _get_program(PRECISION)

    xt = np.ascontiguousarray(x.T)                       # [D, T_FULL]
    xh_full = xt.astype(np.float16)
    xl_full = ((xt - xh_full.astype(np.float32)) * np.float32(XL_SCALE)).astype(F8NP)
    x8_full = xh_full.astype(F8NP)

    wt = np.ascontiguousarray(weight.T)                  # [D, E]
    wt_s = wt * np.float32(S16)
    wh_flat = wt_s.astype(np.float16)
    wl_flat = (wt_s - wh_flat.astype(np.float32)).astype(F8NP)
    w8_flat = (wt * np.float32(W8_SCALE)).astype(F8NP)
    wh_h = _wlayout(wh_flat)
    w8_h = _wlayout(w8_flat)
    wl_h = _wlayout(wl_flat)

    in_maps = [
        {
            "xh": _xlayout(xh_full, c),
            "xl": _xlayout(xl_full, c),
            "x8": _xlayout(x8_full, c),
            "wh": wh_h,
            "w8": w8_h,
            "wl": wl_h,
        }
        for c in range(N_CORES)
    ]
    if _trace:
        import prof

        results, exec_time_ns, percore, neff_dir = prof.profiled_run(
            nc, in_maps, core_ids=list(range(N_CORES))
        )
        _CACHE["last_result"] = {
            "exec_time_ns": exec_time_ns,
            "percore": percore,
            "neff_dir": neff_dir,
        }
    else:
        res = run_bass_kernel_spmd(nc, in_maps, core_ids=list(range(N_CORES)))
        results = res.results
    w_full = np.concatenate([results[c]["w_out"] for c in range(N_CORES)], axis=0)
    i_full = np.concatenate(
        [results[c]["i_out"].astype(np.int32) for c in range(N_CORES)], axis=0
    )
    return w_full, i_full
